# revision 3
# baseline (speedup 1.0000x reference)
"""LocalRNN (windowed GRU, K=16) Trainium2 Bass kernel.

Problem: x [16, 1024, 256] fp32; GRU weights W_ih/W_hh [768, 256|256],
biases [768]. For each position l, run a GRU over the K=16 window
x[l-15 : l+1] (zero left-padded) starting from h=0; output the final
hidden state -> [16, 1024, 256].

Sharding: pure data-parallel over batch: core c gets batch rows
{2c, 2c+1}. No collectives, no halo (window padding is per-row).

Device algorithm (per core, positions on the free axis, hidden units on
partitions; everything fp16 except PSUM/biases):
  - gi = W_ih @ x_padded^T + b_ih  computed ONCE for all window steps
    (each x position appears in K windows; the reference recomputes this
    K times). For r,z rows the evict bias also folds in b_hh (their gate
    math only ever uses i+h summed). gi_n additionally stored shifted by
    one column so fp16 slices stay 4B-aligned for DVE 2x mode at odd t.
  - per window step t and batch row b (jobs split by h-chunk hc):
      psum_r[hc] = W_hh_r @ h + I @ gi_r(t)  (identity-matmul fold on PE)
      psum_z[hc] = W_hh_z @ h + I @ gi_z(t)
      psum_n[hc] = W_hh_n @ h
      r[hc] = sigmoid(psum_r)  z[hc] = sigmoid(psum_z)  [ACT, from PSUM]
      t2[hc] = (psum_n + b_hh_n) * r + gi_n(t)  [DVE stt + add]
      n = tanh(t2)                              [ACT, both hc at once]
      h' = n + z*(h - n)                        [3 DVE ops, both hc at once]
  - step 0 specializes h=0 (no matmuls at all).
"""

import os
import sys
import time
from contextlib import ExitStack

import numpy as np

for _p in (
    "/root/.axon_site",
    "/root/.axon_site/_ro/trn_rl_repo",
    "/root/.axon_site/_ro/pypackages",
):
    if os.path.isdir(_p) and _p not in sys.path:
        sys.path.append(_p)

from concourse import bacc, mybir, tile  # noqa: E402
from concourse.bass_utils import run_bass_kernel_spmd  # noqa: E402

F16 = mybir.dt.float16
F32 = mybir.dt.float32
AF = mybir.ActivationFunctionType
ALU = mybir.AluOpType

B, L, D, H, K = 16, 1024, 256, 256, 16
G3 = 3 * H
NCORES = 8
BPC = B // NCORES  # batch rows per core
DC = D // 128      # contraction chunks over D
HC = H // 128      # contraction chunks over H


def build_program(L=L, K=K, BPC=BPC, num_devices=NCORES):
    """Build + bass-compile the SPMD program (identical on all cores)."""
    PAD = K - 1
    GW = PAD + L + 1  # gi width; +1 keeps it even (unused tail col)
    assert GW % 2 == 0 and L % 512 == 0
    NJ = L // 512  # 512-col matmul regions per psum tile
    L2 = HC * L    # width of hc-merged per-batch tiles

    nc = bacc.Bacc(
        "TRN2",
        target_bir_lowering=False,
        debug=False,
        enable_asserts=False,
        num_devices=num_devices,
    )
    xT = nc.dram_tensor("xT", [DC, 128, BPC, GW], F16, kind="ExternalInput").ap()
    wih = nc.dram_tensor("wih", [DC, 128, G3], F16, kind="ExternalInput").ap()
    whh = nc.dram_tensor("whh", [HC, 128, G3], F16, kind="ExternalInput").ap()
    ident = nc.dram_tensor("ident", [128, 128], F16, kind="ExternalInput").ap()
    # gi evict bias per gate-ptile (col g*HC+hc): b_ih (+ b_hh for r,z)
    gbias = nc.dram_tensor("gbias", [128, 3 * HC], F32, kind="ExternalInput").ap()
    # b_hh for the n gate, per h-chunk
    nbias = nc.dram_tensor("nbias", [128, HC], F32, kind="ExternalInput").ap()
    hout = nc.dram_tensor("hout", [BPC, 128, L2], F16, kind="ExternalOutput").ap()

    with tile.TileContext(nc) as tc, ExitStack() as ctx:
        const = ctx.enter_context(tc.tile_pool(name="const", bufs=1))
        gip = ctx.enter_context(tc.tile_pool(name="gip", bufs=1))
        hp = ctx.enter_context(tc.tile_pool(name="hp", bufs=2))
        work = ctx.enter_context(tc.tile_pool(name="work", bufs=2))

        # ---- resident inputs (batch 0 tiles first so gi can start early) ----
        xt_sb = {}
        for b in range(BPC):
            for dc in range(DC):
                t = const.tile([128, GW], F16, tag=f"xt{dc}{b}", name=f"xt{dc}{b}")
                nc.sync.dma_start(t[:], xT[dc, :, b, :])
                xt_sb[dc, b] = t
        wih_sb = []
        for dc in range(DC):
            t = const.tile([128, G3], F16, tag=f"wih{dc}")
            nc.sync.dma_start(t[:], wih[dc])
            wih_sb.append(t)
        gb_sb = const.tile([128, 3 * HC], F32, tag="gbias")
        nc.sync.dma_start(gb_sb[:], gbias[:])
        nb_sb = const.tile([128, HC], F32, tag="nbias")
        nc.sync.dma_start(nb_sb[:], nbias[:])
        whh_sb = []
        for hc in range(HC):
            t = const.tile([128, G3], F16, tag=f"whh{hc}")
            nc.sync.dma_start(t[:], whh[hc])
            whh_sb.append(t)
        id_sb = const.tile([128, 128], F16, tag="ident")
        nc.sync.dma_start(id_sb[:], ident[:])

        # ---- gi phase: gi[g,hc,b] = W_ih[g-rows] @ xT + bias ----
        gi_sb = {}
        gin_odd = {}
        for g in range(3):
            for hc in range(HC):
                for b in range(BPC):
                    gi_sb[g, hc, b] = gip.tile(
                        [128, GW], F16, tag=f"gi{g}{hc}{b}", name=f"gi{g}{hc}{b}")
                    if g == 2:
                        gin_odd[hc, b] = gip.tile(
                            [128, GW], F16, tag=f"go{hc}{b}", name=f"go{hc}{b}")

        chunks = [(c, min(c + 512, GW)) for c in range(0, GW, 512)]
        with tc.tile_pool(name="pgi", bufs=2, space="PSUM") as pgi:
            ev = 0
            for b in range(BPC):
                for g in range(3):
                    for hc in range(HC):
                        ps = pgi.tile([128, GW], F32, tag="pgi", name="pgi")
                        wcol = g * H + hc * 128
                        for (c0, c1) in chunks:
                            for dc in range(DC):
                                nc.tensor.matmul(
                                    ps[:, c0:c1],
                                    wih_sb[dc][:, wcol:wcol + 128],
                                    xt_sb[dc, b][:, c0:c1],
                                    start=(dc == 0),
                                    stop=(dc == DC - 1),
                                )
                        dst = gi_sb[g, hc, b]
                        bias_ap = gb_sb[:, g * HC + hc:g * HC + hc + 1]
                        # alternate evict engine to split the work
                        if ev % 2 == 0:
                            nc.scalar.activation(dst[:], ps[:], AF.Identity, bias=bias_ap)
                        else:
                            nc.vector.tensor_scalar_add(dst[:], ps[:], bias_ap)
                        ev += 1
                        if g == 2:
                            # shifted copy for 4B-aligned odd-t slices
                            od = gin_odd[hc, b]
                            nc.vector.tensor_copy(od[:, 0:GW - 2], dst[:, 1:GW - 1])

        # ---- recurrence ----
        # h tiles are per-batch [128, HC*L] (hc-chunks side by side)
        hcur = {}

        # step 0: h = 0
        for b in range(BPC):
            nt = work.tile([128, L2], F16, tag=f"n{b}", name=f"n{b}")
            zt = work.tile([128, L2], F16, tag=f"z{b}", name=f"z{b}")
            t2 = work.tile([128, L2], F16, tag=f"t2{b}", name=f"t2{b}")
            for hc in range(HC):
                r = work.tile([128, L], F16, tag=f"r{hc}{b}", name=f"r{hc}{b}")
                nc.scalar.activation(r[:], gi_sb[0, hc, b][:, 0:L], AF.Sigmoid)
                nc.scalar.activation(
                    zt[:, hc * L:(hc + 1) * L], gi_sb[1, hc, b][:, 0:L], AF.Sigmoid)
                nc.vector.scalar_tensor_tensor(
                    t2[:, hc * L:(hc + 1) * L], r[:], nb_sb[:, hc:hc + 1],
                    gi_sb[2, hc, b][:, 0:L], op0=ALU.mult, op1=ALU.add,
                )
            nc.scalar.activation(nt[:], t2[:], AF.Tanh)
            e = work.tile([128, L2], F16, tag=f"d{b}", name=f"d{b}")
            nc.vector.tensor_mul(e[:], nt[:], zt[:])
            h = hp.tile([128, L2], F16, tag=f"h{b}", name=f"h{b}")
            nc.vector.tensor_sub(h[:], nt[:], e[:])
            hcur[b] = h

        with (
            tc.tile_pool(name="prp", bufs=2, space="PSUM") as prp,
            tc.tile_pool(name="pzp", bufs=1, space="PSUM") as pzp,
            tc.tile_pool(name="pnp", bufs=1, space="PSUM") as pnp,
        ):
            for t in range(1, K):
                for b in range(BPC):
                    nt = work.tile([128, L2], F16, tag=f"n{b}", name=f"n{b}")
                    zt = work.tile([128, L2], F16, tag=f"z{b}", name=f"z{b}")
                    t2 = work.tile([128, L2], F16, tag=f"t2{b}", name=f"t2{b}")
                    for hc in range(HC):
                        pr = prp.tile([128, L], F32, tag="pr", name="pr")
                        pz = pzp.tile([128, L], F32, tag="pz", name="pz")
                        pn = pnp.tile([128, L], F32, tag="pn", name="pn")
                        # matmuls: W_hh rows for (gate, hc) against both h chunks
                        for g, ps in ((0, pr), (1, pz), (2, pn)):
                            wcol = g * H + hc * 128
                            for kc in range(HC):
                                for j in range(NJ):
                                    s = slice(j * 512, (j + 1) * 512)
                                    hs = slice(kc * L + j * 512, kc * L + (j + 1) * 512)
                                    nc.tensor.matmul(
                                        ps[:, s],
                                        whh_sb[kc][:, wcol:wcol + 128],
                                        hcur[b][:, hs],
                                        start=(kc == 0),
                                        stop=(kc == HC - 1 and g == 2),
                                    )
                            if g < 2:
                                for j in range(NJ):
                                    s = slice(j * 512, (j + 1) * 512)
                                    nc.tensor.matmul(
                                        ps[:, s],
                                        id_sb[:],
                                        gi_sb[g, hc, b][:, t + j * 512:t + (j + 1) * 512],
                                        start=False,
                                        stop=True,
                                    )
                        r = work.tile([128, L], F16, tag=f"r{hc}{b}", name=f"r{hc}{b}")
                        nc.scalar.activation(r[:], pr[:], AF.Sigmoid)
                        nc.scalar.activation(zt[:, hc * L:(hc + 1) * L], pz[:], AF.Sigmoid)
                        t1 = work.tile([128, L], F16, tag=f"t1{hc}{b}", name=f"t1{hc}{b}")
                        nc.vector.scalar_tensor_tensor(
                            t1[:], pn[:], nb_sb[:, hc:hc + 1], r[:],
                            op0=ALU.add, op1=ALU.mult,
                        )
                        if t % 2 == 0:
                            gin, off = gi_sb[2, hc, b], t
                        else:
                            gin, off = gin_odd[hc, b], t - 1
                        nc.vector.tensor_add(
                            t2[:, hc * L:(hc + 1) * L], t1[:], gin[:, off:off + L])
                    nc.scalar.activation(nt[:], t2[:], AF.Tanh)
                    d = work.tile([128, L2], F16, tag=f"d{b}", name=f"d{b}")
                    nc.vector.tensor_sub(d[:], hcur[b][:], nt[:])
                    nc.vector.tensor_mul(d[:], d[:], zt[:])
                    hnew = hp.tile([128, L2], F16, tag=f"h{b}", name=f"h{b}")
                    nc.vector.tensor_add(hnew[:], nt[:], d[:])
                    hcur[b] = hnew
                    if t == K - 1:
                        nc.sync.dma_start(hout[b], hnew[:])

    nc.compile()
    return nc


def make_inputs(x, W_ih, W_hh, b_ih, b_hh, L=L, K=K, BPC=BPC, ncores=NCORES):
    """Host-side shard + layout prep. Returns in_maps for run_bass_kernel_spmd."""
    PAD = K - 1
    GW = PAD + L + 1
    wih_in = np.ascontiguousarray(
        W_ih.astype(np.float16).T.reshape(DC, 128, G3))
    whh_in = np.ascontiguousarray(
        W_hh.astype(np.float16).T.reshape(HC, 128, G3))
    ident = np.eye(128, dtype=np.float16)
    gbias = np.zeros((128, 3 * HC), np.float32)
    nbias = np.zeros((128, HC), np.float32)
    for hc in range(HC):
        for g in range(3):
            v = b_ih[g * H + hc * 128:g * H + (hc + 1) * 128].astype(np.float32)
            if g < 2:
                v = v + b_hh[g * H + hc * 128:g * H + (hc + 1) * 128]
            gbias[:, g * HC + hc] = v
        nbias[:, hc] = b_hh[2 * H + hc * 128:2 * H + (hc + 1) * 128]

    in_maps = []
    for c in range(ncores):
        xTc = np.zeros((DC, 128, BPC, GW), np.float16)
        for b in range(BPC):
            xb = x[c * BPC + b].astype(np.float16).T  # [D, L]
            for dc in range(DC):
                xTc[dc, :, b, PAD:PAD + L] = xb[dc * 128:(dc + 1) * 128]
        in_maps.append({
            "xT": xTc, "wih": wih_in, "whh": whh_in, "ident": ident,
            "gbias": gbias, "nbias": nbias,
        })
    return in_maps


def gather_output(results, L=L, BPC=BPC, ncores=NCORES):
    out = np.empty((ncores * BPC, L, H), np.float32)
    for c in range(ncores):
        ho = results[c]["hout"]  # [BPC, 128, HC*L] f16
        for b in range(BPC):
            for hc in range(HC):
                out[c * BPC + b, :, hc * 128:(hc + 1) * 128] = \
                    ho[b, :, hc * L:(hc + 1) * L].T
    return out


_prog = None


def _get_program():
    global _prog
    if _prog is None:
        _prog = build_program()
    return _prog


def kernel(x, W_ih, W_hh, b_ih, b_hh):
    x = np.asarray(x)
    nc = _get_program()
    in_maps = make_inputs(np.asarray(x), np.asarray(W_ih), np.asarray(W_hh),
                          np.asarray(b_ih), np.asarray(b_hh))
    res = run_bass_kernel_spmd(nc, in_maps, core_ids=list(range(NCORES)))
    return gather_output(res.results)


if __name__ == "__main__":
    t0 = time.time()
    _get_program()
    print(f"build+compile: {time.time() - t0:.1f}s")


# revision 6
# speedup vs baseline: 1.4896x; 1.4896x over previous
"""LocalRNN (windowed GRU, K=16) Trainium2 Bass kernel.

Problem: x [16, 1024, 256] fp32; GRU weights W_ih/W_hh [768, 256|256],
biases [768]. For each position l, run a GRU over the K=16 window
x[l-15 : l+1] (zero left-padded) starting from h=0; output the final
hidden state -> [16, 1024, 256].

Sharding: pure data-parallel over batch: core c gets batch rows
{2c, 2c+1}. No collectives, no halo (window padding is per-row).

Device algorithm (per core, positions npos = 2*1024 on the free axis,
hidden units on partitions):
  - gi = W_ih @ x_padded^T + b_ih  computed ONCE for all window steps
    (each x position appears in K windows; the reference recomputes this
    K times). Stored fp16 in SBUF, [768 rows -> 6 ptiles, 1040 cols/row].
    For r,z rows the evict bias also folds in b_hh (their gate math only
    ever uses i+h summed). gi_n additionally stored shifted by one
    column so fp16 slices stay 4B-aligned for DVE 2x mode at odd t.
  - per window step t (h != 0 steps):
      psum_r = W_hh_r @ h + I @ gi_r(t)   (identity-matmul fold on PE)
      psum_z = W_hh_z @ h + I @ gi_z(t)
      psum_n = W_hh_n @ h
      r = sigmoid(psum_r)  z = sigmoid(psum_z)          [ACT, from PSUM]
      t1 = (psum_n + b_hh_n) * r                        [DVE scalar_tensor_tensor]
      t2 = t1 + gi_n(t) ; n = tanh(t2)                  [DVE + ACT]
      d = h - n                                         [GPSIMD]
      h' = n + z*d                                      [DVE]
  - step 0 specializes h=0 (no matmuls at all).
"""

import os
import sys
import time
from contextlib import ExitStack

import numpy as np

for _p in (
    "/root/.axon_site",
    "/root/.axon_site/_ro/trn_rl_repo",
    "/root/.axon_site/_ro/pypackages",
):
    if os.path.isdir(_p) and _p not in sys.path:
        sys.path.append(_p)

from concourse import bacc, mybir, tile  # noqa: E402
from concourse.bass_utils import run_bass_kernel_spmd  # noqa: E402

F16 = mybir.dt.float16
F32 = mybir.dt.float32
AF = mybir.ActivationFunctionType
ALU = mybir.AluOpType

B, L, D, H, K = 16, 1024, 256, 256, 16
G3 = 3 * H
NCORES = 8
BPC = B // NCORES  # batch rows per core
DC = D // 128      # contraction chunks over D
HC = H // 128      # contraction chunks over H


def build_program(L=L, K=K, BPC=BPC, num_devices=NCORES):
    """Build + bass-compile the SPMD program (identical on all cores)."""
    PAD = K - 1
    GW = PAD + L + 1  # gi width; +1 keeps it even (unused tail col)
    assert GW % 2 == 0 and L % 512 == 0
    NJ = L // 512  # 512-col matmul regions per job

    nc = bacc.Bacc(
        "TRN2",
        target_bir_lowering=False,
        debug=False,
        enable_asserts=False,
        num_devices=num_devices,
    )
    xT = nc.dram_tensor("xT", [DC, 128, BPC, GW], F16, kind="ExternalInput").ap()
    wih = nc.dram_tensor("wih", [DC, 128, G3], F16, kind="ExternalInput").ap()
    whh = nc.dram_tensor("whh", [HC, 128, G3], F16, kind="ExternalInput").ap()
    ident = nc.dram_tensor("ident", [128, 128], F16, kind="ExternalInput").ap()
    # gi evict bias per gate-ptile (col g*HC+hc): b_ih (+ b_hh for r,z)
    gbias = nc.dram_tensor("gbias", [128, 3 * HC], F32, kind="ExternalInput").ap()
    # b_hh for the n gate, per h-chunk
    nbias = nc.dram_tensor("nbias", [128, HC], F32, kind="ExternalInput").ap()
    hout = nc.dram_tensor("hout", [HC, 128, BPC, L], F16, kind="ExternalOutput").ap()

    with tile.TileContext(nc) as tc, ExitStack() as ctx:
        const = ctx.enter_context(tc.tile_pool(name="const", bufs=1))
        gip = ctx.enter_context(tc.tile_pool(name="gip", bufs=1))
        hp = ctx.enter_context(tc.tile_pool(name="hp", bufs=2))
        work = ctx.enter_context(tc.tile_pool(name="work", bufs=2))

        # ---- resident inputs (weights + batch-0 x first so gi starts early) ----
        wih_sb = []
        for dc in range(DC):
            t = const.tile([128, G3], F16, tag=f"wih{dc}")
            nc.sync.dma_start(t[:], wih[dc])
            wih_sb.append(t)
        xt_sb = {}
        for b in range(BPC):
            for dc in range(DC):
                t = const.tile([128, GW], F16, tag=f"xt{dc}{b}", name=f"xt{dc}{b}")
                nc.sync.dma_start(t[:], xT[dc, :, b, :])
                xt_sb[dc, b] = t
        gb_sb = const.tile([128, 3 * HC], F32, tag="gbias")
        nc.sync.dma_start(gb_sb[:], gbias[:])
        nb_sb = const.tile([128, HC], F32, tag="nbias")
        nc.sync.dma_start(nb_sb[:], nbias[:])
        whh_sb = []
        for hc in range(HC):
            t = const.tile([128, G3], F16, tag=f"whh{hc}")
            nc.sync.dma_start(t[:], whh[hc])
            whh_sb.append(t)
        id_sb = const.tile([128, 128], F16, tag="ident")
        nc.sync.dma_start(id_sb[:], ident[:])

        # ---- gi phase: gi[g,hc,b] = W_ih[g-rows] @ xT + bias ----
        gi_sb = {}
        gin_odd = {}
        for g in range(3):
            for hc in range(HC):
                for b in range(BPC):
                    gi_sb[g, hc, b] = gip.tile([128, GW], F16, tag=f"gi{g}{hc}{b}", name=f"gi{g}{hc}{b}")
                    if g == 2:
                        gin_odd[hc, b] = gip.tile([128, GW], F16, tag=f"go{hc}{b}", name=f"go{hc}{b}")

        chunks = [(c, min(c + 512, GW)) for c in range(0, GW, 512)]
        with tc.tile_pool(name="pgi", bufs=2, space="PSUM") as pgi:
            ev = 0
            for b in range(BPC):
                for g in range(3):
                    for hc in range(HC):
                        ps = pgi.tile([128, GW], F32, tag="pgi", name="pgi")
                        wcol = g * H + hc * 128
                        for (c0, c1) in chunks:
                            for dc in range(DC):
                                nc.tensor.matmul(
                                    ps[:, c0:c1],
                                    wih_sb[dc][:, wcol:wcol + 128],
                                    xt_sb[dc, b][:, c0:c1],
                                    start=(dc == 0),
                                    stop=(dc == DC - 1),
                                )
                        dst = gi_sb[g, hc, b]
                        bias_ap = gb_sb[:, g * HC + hc:g * HC + hc + 1]
                        # alternate evict engine to split the work
                        if ev % 2 == 0:
                            nc.scalar.activation(dst[:], ps[:], AF.Identity, bias=bias_ap)
                        else:
                            nc.vector.tensor_scalar_add(dst[:], ps[:], bias_ap)
                        ev += 1
                        if g == 2:
                            # shifted copy for 4B-aligned odd-t slices
                            od = gin_odd[hc, b]
                            nc.vector.tensor_copy(od[:, 0:GW - 2], dst[:, 1:GW - 1])

        # ---- recurrence ----
        hcur = {}

        # step 0: h = 0
        for b in range(BPC):
            for hc in range(HC):
                r = work.tile([128, L], F16, tag=f"r{hc}{b}")
                nc.scalar.activation(r[:], gi_sb[0, hc, b][:, 0:L], AF.Sigmoid)
                z = work.tile([128, L], F16, tag=f"z{hc}{b}")
                nc.scalar.activation(z[:], gi_sb[1, hc, b][:, 0:L], AF.Sigmoid)
                t2 = work.tile([128, L], F16, tag=f"t1{hc}{b}")
                nc.vector.scalar_tensor_tensor(
                    t2[:], r[:], nb_sb[:, hc:hc + 1], gi_sb[2, hc, b][:, 0:L],
                    op0=ALU.mult, op1=ALU.add,
                )
                n = work.tile([128, L], F16, tag=f"n{hc}{b}")
                nc.scalar.activation(n[:], t2[:], AF.Tanh)
                e = work.tile([128, L], F16, tag=f"d{hc}{b}")
                nc.vector.tensor_mul(e[:], n[:], z[:])
                h = hp.tile([128, L], F16, tag=f"h{hc}{b}")
                nc.vector.tensor_sub(h[:], n[:], e[:])
                hcur[hc, b] = h

        with (
            tc.tile_pool(name="prp", bufs=2, space="PSUM") as prp,
            tc.tile_pool(name="pzp", bufs=1, space="PSUM") as pzp,
            tc.tile_pool(name="pnp", bufs=1, space="PSUM") as pnp,
        ):
            for t in range(1, K):
                for b in range(BPC):
                    hnext = {}
                    for hc in range(HC):
                        pr = prp.tile([128, L], F32, tag="pr", name="pr")
                        pz = pzp.tile([128, L], F32, tag="pz", name="pz")
                        pn = pnp.tile([128, L], F32, tag="pn", name="pn")
                        # matmuls: W_hh rows for (gate, hc) against both h chunks
                        for g, ps in ((0, pr), (1, pz), (2, pn)):
                            wcol = g * H + hc * 128
                            for kc in range(HC):
                                for j in range(NJ):
                                    s = slice(j * 512, (j + 1) * 512)
                                    nc.tensor.matmul(
                                        ps[:, s],
                                        whh_sb[kc][:, wcol:wcol + 128],
                                        hcur[kc, b][:, s],
                                        start=(kc == 0),
                                        stop=(kc == HC - 1 and g == 2),
                                    )
                            if g < 2:
                                for j in range(NJ):
                                    s = slice(j * 512, (j + 1) * 512)
                                    nc.tensor.matmul(
                                        ps[:, s],
                                        id_sb[:],
                                        gi_sb[g, hc, b][:, t + j * 512:t + (j + 1) * 512],
                                        start=False,
                                        stop=True,
                                    )
                        r = work.tile([128, L], F16, tag=f"r{hc}{b}")
                        nc.scalar.activation(r[:], pr[:], AF.Sigmoid)
                        z = work.tile([128, L], F16, tag=f"z{hc}{b}")
                        nc.scalar.activation(z[:], pz[:], AF.Sigmoid)
                        t1 = work.tile([128, L], F16, tag=f"t1{hc}{b}")
                        nc.vector.scalar_tensor_tensor(
                            t1[:], pn[:], nb_sb[:, hc:hc + 1], r[:],
                            op0=ALU.add, op1=ALU.mult,
                        )
                        if t % 2 == 0:
                            gin, off = gi_sb[2, hc, b], t
                        else:
                            gin, off = gin_odd[hc, b], t - 1
                        nc.vector.tensor_add(t1[:], t1[:], gin[:, off:off + L])
                        n = work.tile([128, L], F16, tag=f"n{hc}{b}")
                        nc.scalar.activation(n[:], t1[:], AF.Tanh)
                        d = work.tile([128, L], F16, tag=f"d{hc}{b}")
                        nc.vector.tensor_sub(d[:], hcur[hc, b][:], n[:])
                        nc.vector.tensor_mul(d[:], d[:], z[:])
                        hnew = hp.tile([128, L], F16, tag=f"h{hc}{b}")
                        nc.vector.tensor_add(hnew[:], n[:], d[:])
                        hnext[hc] = hnew
                    for hc in range(HC):
                        hcur[hc, b] = hnext[hc]

        for hc in range(HC):
            for b in range(BPC):
                nc.sync.dma_start(hout[hc, :, b, :], hcur[hc, b][:])

    nc.compile()
    return nc


def make_inputs(x, W_ih, W_hh, b_ih, b_hh, L=L, K=K, BPC=BPC, ncores=NCORES):
    """Host-side shard + layout prep. Returns in_maps for run_bass_kernel_spmd."""
    PAD = K - 1
    GW = PAD + L + 1
    wih_in = np.ascontiguousarray(
        W_ih.astype(np.float16).T.reshape(DC, 128, G3))
    whh_in = np.ascontiguousarray(
        W_hh.astype(np.float16).T.reshape(HC, 128, G3))
    ident = np.eye(128, dtype=np.float16)
    gbias = np.zeros((128, 3 * HC), np.float32)
    nbias = np.zeros((128, HC), np.float32)
    for hc in range(HC):
        for g in range(3):
            v = b_ih[g * H + hc * 128:g * H + (hc + 1) * 128].astype(np.float32)
            if g < 2:
                v = v + b_hh[g * H + hc * 128:g * H + (hc + 1) * 128]
            gbias[:, g * HC + hc] = v
        nbias[:, hc] = b_hh[2 * H + hc * 128:2 * H + (hc + 1) * 128]

    in_maps = []
    for c in range(ncores):
        xTc = np.zeros((DC, 128, BPC, GW), np.float16)
        for b in range(BPC):
            xb = x[c * BPC + b].astype(np.float16).T  # [D, L]
            for dc in range(DC):
                xTc[dc, :, b, PAD:PAD + L] = xb[dc * 128:(dc + 1) * 128]
        in_maps.append({
            "xT": xTc, "wih": wih_in, "whh": whh_in, "ident": ident,
            "gbias": gbias, "nbias": nbias,
        })
    return in_maps


def gather_output(results, L=L, BPC=BPC, ncores=NCORES):
    out = np.empty((ncores * BPC, L, H), np.float32)
    for c in range(ncores):
        ho = results[c]["hout"]  # [HC, 128, BPC, L] f16
        for b in range(BPC):
            for hc in range(HC):
                out[c * BPC + b, :, hc * 128:(hc + 1) * 128] = ho[hc, :, b, :].T
    return out


_prog = None


def _get_program():
    global _prog
    if _prog is None:
        _prog = build_program()
    return _prog


def kernel(x, W_ih, W_hh, b_ih, b_hh):
    x = np.asarray(x)
    nc = _get_program()
    in_maps = make_inputs(np.asarray(x), np.asarray(W_ih), np.asarray(W_hh),
                          np.asarray(b_ih), np.asarray(b_hh))
    res = run_bass_kernel_spmd(nc, in_maps, core_ids=list(range(NCORES)))
    return gather_output(res.results)


if __name__ == "__main__":
    t0 = time.time()
    _get_program()
    print(f"build+compile: {time.time() - t0:.1f}s")


# revision 10
# speedup vs baseline: 1.4966x; 1.0047x over previous
"""LocalRNN (windowed GRU, K=16) Trainium2 Bass kernel.

Problem: x [16, 1024, 256] fp32; GRU weights W_ih/W_hh [768, 256|256],
biases [768]. For each position l, run a GRU over the K=16 window
x[l-15 : l+1] (zero left-padded) starting from h=0; output the final
hidden state -> [16, 1024, 256].

Sharding: pure data-parallel over batch: core c gets batch rows
{2c, 2c+1}. No collectives, no halo (window padding is per-row).

Device algorithm (per core, positions npos = 2*1024 on the free axis,
hidden units on partitions):
  - gi = W_ih @ x_padded^T + b_ih  computed ONCE for all window steps
    (each x position appears in K windows; the reference recomputes this
    K times). Stored fp16 in SBUF, [768 rows -> 6 ptiles, 1040 cols/row].
    For r,z rows the evict bias also folds in b_hh (their gate math only
    ever uses i+h summed). gi_n additionally stored shifted by one
    column so fp16 slices stay 4B-aligned for DVE 2x mode at odd t.
  - per window step t (h != 0 steps):
      psum_r = W_hh_r @ h + I @ gi_r(t)   (identity-matmul fold on PE)
      psum_z = W_hh_z @ h + I @ gi_z(t)
      psum_n = W_hh_n @ h
      r = sigmoid(psum_r)  z = sigmoid(psum_z)          [ACT, from PSUM]
      t1 = (psum_n + b_hh_n) * r                        [DVE scalar_tensor_tensor]
      t2 = t1 + gi_n(t) ; n = tanh(t2)                  [DVE + ACT]
      d = h - n                                         [GPSIMD]
      h' = n + z*d                                      [DVE]
  - step 0 specializes h=0 (no matmuls at all).
"""

import os
import sys
import time
from contextlib import ExitStack

import numpy as np

for _p in (
    "/root/.axon_site",
    "/root/.axon_site/_ro/trn_rl_repo",
    "/root/.axon_site/_ro/pypackages",
):
    if os.path.isdir(_p) and _p not in sys.path:
        sys.path.append(_p)

from concourse import bacc, mybir, tile  # noqa: E402
from concourse.bass_utils import run_bass_kernel_spmd  # noqa: E402

F16 = mybir.dt.float16
F32 = mybir.dt.float32
AF = mybir.ActivationFunctionType
ALU = mybir.AluOpType

B, L, D, H, K = 16, 1024, 256, 256, 16
G3 = 3 * H
NCORES = 8
BPC = B // NCORES  # batch rows per core
DC = D // 128      # contraction chunks over D
HC = H // 128      # contraction chunks over H


def build_program(L=L, K=K, BPC=BPC, num_devices=NCORES):
    """Build + bass-compile the SPMD program (identical on all cores)."""
    PAD = K - 1
    GW = PAD + L + 1  # gi width; +1 keeps it even (unused tail col)
    assert GW % 2 == 0 and L % 512 == 0
    NJ = L // 512  # 512-col matmul regions per job

    nc = bacc.Bacc(
        "TRN2",
        target_bir_lowering=False,
        debug=False,
        enable_asserts=False,
        num_devices=num_devices,
    )
    xT = nc.dram_tensor("xT", [DC, 128, BPC, GW], F16, kind="ExternalInput").ap()
    wih = nc.dram_tensor("wih", [DC, 128, G3], F16, kind="ExternalInput").ap()
    whh = nc.dram_tensor("whh", [HC, 128, G3], F16, kind="ExternalInput").ap()
    ident = nc.dram_tensor("ident", [128, 128], F16, kind="ExternalInput").ap()
    # gi evict bias per gate-ptile (col g*HC+hc): b_ih (+ b_hh for r,z)
    gbias = nc.dram_tensor("gbias", [128, 3 * HC], F32, kind="ExternalInput").ap()
    # b_hh for the n gate, per h-chunk
    nbias = nc.dram_tensor("nbias", [128, HC], F32, kind="ExternalInput").ap()
    hout = nc.dram_tensor("hout", [HC, 128, BPC, L], F16, kind="ExternalOutput").ap()

    with tile.TileContext(nc) as tc, ExitStack() as ctx:
        const = ctx.enter_context(tc.tile_pool(name="const", bufs=1))
        gip = ctx.enter_context(tc.tile_pool(name="gip", bufs=1))
        hp = ctx.enter_context(tc.tile_pool(name="hp", bufs=2))
        work = ctx.enter_context(tc.tile_pool(name="work", bufs=2))

        # ---- resident inputs (weights + batch-0 x first so gi starts early) ----
        wih_sb = []
        for dc in range(DC):
            t = const.tile([128, G3], F16, tag=f"wih{dc}")
            nc.sync.dma_start(t[:], wih[dc])
            wih_sb.append(t)
        xt_sb = {}
        for b in range(BPC):
            for dc in range(DC):
                t = const.tile([128, GW], F16, tag=f"xt{dc}{b}", name=f"xt{dc}{b}")
                # split the big loads across two DMA queues
                (nc.sync if dc == 0 else nc.gpsimd).dma_start(t[:], xT[dc, :, b, :])
                xt_sb[dc, b] = t
        gb_sb = const.tile([128, 3 * HC], F32, tag="gbias")
        nc.sync.dma_start(gb_sb[:], gbias[:])
        nb_sb = const.tile([128, HC], F32, tag="nbias")
        nc.sync.dma_start(nb_sb[:], nbias[:])
        whh_sb = []
        for hc in range(HC):
            t = const.tile([128, G3], F16, tag=f"whh{hc}")
            nc.gpsimd.dma_start(t[:], whh[hc])
            whh_sb.append(t)
        id_sb = const.tile([128, 128], F16, tag="ident")
        nc.gpsimd.dma_start(id_sb[:], ident[:])

        # ---- gi phase: gi[g,hc,b] = W_ih[g-rows] @ xT + bias ----
        gi_sb = {}
        gin_odd = {}
        for g in range(3):
            for hc in range(HC):
                for b in range(BPC):
                    gi_sb[g, hc, b] = gip.tile([128, GW], F16, tag=f"gi{g}{hc}{b}", name=f"gi{g}{hc}{b}")
                    if g == 2:
                        gin_odd[hc, b] = gip.tile([128, GW], F16, tag=f"go{hc}{b}", name=f"go{hc}{b}")

        chunks = [(c, min(c + 512, GW)) for c in range(0, GW, 512)]
        with tc.tile_pool(name="pgi", bufs=2, space="PSUM") as pgi:
            ev = 0
            for b in range(BPC):
                for g in range(3):
                    for hc in range(HC):
                        ps = pgi.tile([128, GW], F32, tag="pgi", name="pgi")
                        wcol = g * H + hc * 128
                        for (c0, c1) in chunks:
                            for dc in range(DC):
                                nc.tensor.matmul(
                                    ps[:, c0:c1],
                                    wih_sb[dc][:, wcol:wcol + 128],
                                    xt_sb[dc, b][:, c0:c1],
                                    start=(dc == 0),
                                    stop=(dc == DC - 1),
                                )
                        dst = gi_sb[g, hc, b]
                        bias_ap = gb_sb[:, g * HC + hc:g * HC + hc + 1]
                        # alternate evict engine to split the work
                        if ev % 2 == 0:
                            nc.scalar.activation(dst[:], ps[:], AF.Identity, bias=bias_ap)
                        else:
                            nc.vector.tensor_scalar_add(dst[:], ps[:], bias_ap)
                        ev += 1
                        if g == 2:
                            # shifted copy for 4B-aligned odd-t slices
                            od = gin_odd[hc, b]
                            nc.vector.tensor_copy(od[:, 0:GW - 2], dst[:, 1:GW - 1])

        # ---- recurrence ----
        hcur = {}

        # step 0: h = 0
        for b in range(BPC):
            for hc in range(HC):
                r = work.tile([128, L], F16, tag=f"r{hc}{b}")
                nc.scalar.activation(r[:], gi_sb[0, hc, b][:, 0:L], AF.Sigmoid)
                z = work.tile([128, L], F16, tag=f"z{hc}{b}")
                nc.scalar.activation(z[:], gi_sb[1, hc, b][:, 0:L], AF.Sigmoid)
                t2 = work.tile([128, L], F16, tag=f"t1{hc}{b}")
                nc.vector.scalar_tensor_tensor(
                    t2[:], r[:], nb_sb[:, hc:hc + 1], gi_sb[2, hc, b][:, 0:L],
                    op0=ALU.mult, op1=ALU.add,
                )
                n = work.tile([128, L], F16, tag=f"n{hc}{b}")
                nc.scalar.activation(n[:], t2[:], AF.Tanh)
                e = work.tile([128, L], F16, tag=f"d{hc}{b}")
                nc.vector.tensor_mul(e[:], n[:], z[:])
                h = hp.tile([128, L], F16, tag=f"h{hc}{b}")
                nc.vector.tensor_sub(h[:], n[:], e[:])
                hcur[hc, b] = h

        with (
            tc.tile_pool(name="prp", bufs=2, space="PSUM") as prp,
            tc.tile_pool(name="pzp", bufs=1, space="PSUM") as pzp,
            tc.tile_pool(name="pnp", bufs=1, space="PSUM") as pnp,
        ):
            for t in range(1, K):
                for b in range(BPC):
                    hnext = {}
                    for hc in range(HC):
                        pr = prp.tile([128, L], F32, tag="pr", name="pr")
                        pz = pzp.tile([128, L], F32, tag="pz", name="pz")
                        pn = pnp.tile([128, L], F32, tag="pn", name="pn")
                        # identity-folds first: they do not depend on h, so the
                        # PE can run them while the previous step's chain drains
                        for g, ps in ((0, pr), (1, pz)):
                            for j in range(NJ):
                                s = slice(j * 512, (j + 1) * 512)
                                nc.tensor.matmul(
                                    ps[:, s],
                                    id_sb[:],
                                    gi_sb[g, hc, b][:, t + j * 512:t + (j + 1) * 512],
                                    start=True,
                                    stop=False,
                                )
                        # W_hh matmuls for (gate, hc) against both h chunks
                        for g, ps in ((0, pr), (1, pz), (2, pn)):
                            wcol = g * H + hc * 128
                            for kc in range(HC):
                                for j in range(NJ):
                                    s = slice(j * 512, (j + 1) * 512)
                                    nc.tensor.matmul(
                                        ps[:, s],
                                        whh_sb[kc][:, wcol:wcol + 128],
                                        hcur[kc, b][:, s],
                                        start=(kc == 0 and g == 2),
                                        stop=(kc == HC - 1),
                                    )
                        r = work.tile([128, L], F16, tag=f"r{hc}{b}")
                        nc.scalar.activation(r[:], pr[:], AF.Sigmoid)
                        z = work.tile([128, L], F16, tag=f"z{hc}{b}")
                        nc.scalar.activation(z[:], pz[:], AF.Sigmoid)
                        t1 = work.tile([128, L], F16, tag=f"t1{hc}{b}")
                        nc.vector.scalar_tensor_tensor(
                            t1[:], pn[:], nb_sb[:, hc:hc + 1], r[:],
                            op0=ALU.add, op1=ALU.mult,
                        )
                        if t % 2 == 0:
                            gin, off = gi_sb[2, hc, b], t
                        else:
                            gin, off = gin_odd[hc, b], t - 1
                        nc.vector.tensor_add(t1[:], t1[:], gin[:, off:off + L])
                        n = work.tile([128, L], F16, tag=f"n{hc}{b}")
                        nc.scalar.activation(n[:], t1[:], AF.Tanh)
                        d = work.tile([128, L], F16, tag=f"d{hc}{b}")
                        nc.vector.tensor_sub(d[:], hcur[hc, b][:], n[:])
                        nc.vector.tensor_mul(d[:], d[:], z[:])
                        hnew = hp.tile([128, L], F16, tag=f"h{hc}{b}")
                        nc.vector.tensor_add(hnew[:], n[:], d[:])
                        hnext[hc] = hnew
                        if t == K - 1:
                            (nc.sync if hc == 0 else nc.gpsimd).dma_start(
                                hout[hc, :, b, :], hnew[:])
                    for hc in range(HC):
                        hcur[hc, b] = hnext[hc]

    nc.compile()
    return nc


def make_inputs(x, W_ih, W_hh, b_ih, b_hh, L=L, K=K, BPC=BPC, ncores=NCORES):
    """Host-side shard + layout prep. Returns in_maps for run_bass_kernel_spmd."""
    PAD = K - 1
    GW = PAD + L + 1
    wih_in = np.ascontiguousarray(
        W_ih.astype(np.float16).T.reshape(DC, 128, G3))
    whh_in = np.ascontiguousarray(
        W_hh.astype(np.float16).T.reshape(HC, 128, G3))
    ident = np.eye(128, dtype=np.float16)
    gbias = np.zeros((128, 3 * HC), np.float32)
    nbias = np.zeros((128, HC), np.float32)
    for hc in range(HC):
        for g in range(3):
            v = b_ih[g * H + hc * 128:g * H + (hc + 1) * 128].astype(np.float32)
            if g < 2:
                v = v + b_hh[g * H + hc * 128:g * H + (hc + 1) * 128]
            gbias[:, g * HC + hc] = v
        nbias[:, hc] = b_hh[2 * H + hc * 128:2 * H + (hc + 1) * 128]

    in_maps = []
    for c in range(ncores):
        xTc = np.zeros((DC, 128, BPC, GW), np.float16)
        for b in range(BPC):
            xb = x[c * BPC + b].astype(np.float16).T  # [D, L]
            for dc in range(DC):
                xTc[dc, :, b, PAD:PAD + L] = xb[dc * 128:(dc + 1) * 128]
        in_maps.append({
            "xT": xTc, "wih": wih_in, "whh": whh_in, "ident": ident,
            "gbias": gbias, "nbias": nbias,
        })
    return in_maps


def gather_output(results, L=L, BPC=BPC, ncores=NCORES):
    out = np.empty((ncores * BPC, L, H), np.float32)
    for c in range(ncores):
        ho = results[c]["hout"]  # [HC, 128, BPC, L] f16
        for b in range(BPC):
            for hc in range(HC):
                out[c * BPC + b, :, hc * 128:(hc + 1) * 128] = ho[hc, :, b, :].T
    return out


_prog = None


def _get_program():
    global _prog
    if _prog is None:
        _prog = build_program()
    return _prog


def kernel(x, W_ih, W_hh, b_ih, b_hh):
    x = np.asarray(x)
    nc = _get_program()
    in_maps = make_inputs(np.asarray(x), np.asarray(W_ih), np.asarray(W_hh),
                          np.asarray(b_ih), np.asarray(b_hh))
    res = run_bass_kernel_spmd(nc, in_maps, core_ids=list(range(NCORES)))
    return gather_output(res.results)


if __name__ == "__main__":
    t0 = time.time()
    _get_program()
    print(f"build+compile: {time.time() - t0:.1f}s")
